# revision 1
# baseline (speedup 1.0000x reference)
"""DiagonalBiLSTM Trainium2 kernel.

Full inputs in, full output out. Internally: 8-way data-parallel over the
1024 flattened (batch, height) scan rows; both scan directions fused into
one moving dimension (N=508) so fp32r matmuls run at full rate. The
cross-core h_next row coupling is handled with 126 redundant ghost rows
per core (no inter-core communication).

Math per diagonal step d (per row r, channel vector form):
    u  = Wm @ x_diag[d] + k0 @ h[r] + k1 @ h[r+1] + (b_i2s + b_s2s)
    g  = w_ih @ u + (b_ih + b_hh)          # 4*256 gate channels
    c  = sig(g_f)*c + sig(g_i)*tanh(g_g)
    h  = sig(g_o)*tanh(c)
"""

import numpy as np

B, CIN, COUT, H, W, DC = 16, 256, 256, 64, 64, 3
WD = 2 * W - 1          # 127 diagonal steps
NCORES = 8
OWN = 128               # own rows per core (2 batches)
GHOST = 126             # redundant ghost rows
NDATA = OWN + GHOST     # 254 data cols per direction
NCOL = 256              # padded col count per dir (col 254 = always zero)
NKC = 2                 # channel chunks (256 = 2*128)
NGT = 8                 # gate m-tiles (1024 = 8*128)

_COMPILED = {}


# ----------------------------------------------------------------- host prep

def _i2s_mask_np():
    oc = np.arange(COUT) % DC
    ic = np.arange(CIN) % DC
    return (ic[None, :] <= oc[:, None]).astype(np.float32)


def _wT_tiles(w, nmt):
    # [out=nmt*128, in=256] -> lhsT tile array [k=128, kc=2, mt=nmt, m=128]
    return np.ascontiguousarray(
        w.T.reshape(NKC, 128, nmt, 128).transpose(1, 0, 2, 3))


def _diag_pack(x_loc):
    """x_loc [4, 256, 64, 64] (local batches, already W-flipped for the R dir)
    -> [WD, NKC, 128, 256cols] where col j = local row (b_loc*64 + h),
    value x[b, c, h, d - h] (0 outside the diagonal band)."""
    xs = np.zeros((WD, 4, CIN, H), np.float32)
    for h in range(H):
        # diag d = h + w for w in 0..63  ->  xs[h:h+64, :, :, h]
        xs[h:h + W, :, :, h] = x_loc[:, :, h, :].transpose(2, 0, 1)
    # [WD, 4b, 256c, 64h] -> [WD, 256c, 4b*64h] -> [WD, 2, 128, 256]
    xs = xs.transpose(0, 2, 1, 3).reshape(WD, CIN, 4 * H)
    return np.ascontiguousarray(xs.reshape(WD, NKC, 128, 4 * H))


def _prep_inputs(x, w_i2s, b_i2s, w_ih, b_ih, b_hh, k0, k1, b_s2s):
    wm = w_i2s * _i2s_mask_np()
    wm_t = _wT_tiles(wm, 2)
    k0_t = _wT_tiles(k0, 2)
    k1_t = _wT_tiles(k1, 2)
    wih_t = _wT_tiles(w_ih, 8)

    bias_u = (b_i2s + b_s2s).astype(np.float32)       # [256]
    bias_g = (b_ih + b_hh).astype(np.float32)         # [1024]
    misc_all = np.zeros((NCORES, 128, 12), np.float32)
    misc_all[:, :, 0:2] = bias_u.reshape(2, 128).T[None]
    misc_all[:, :, 2:10] = bias_g.reshape(8, 128).T[None]
    misc_all[:, :, 10] = 1.0
    misc_all[7, :, 10] = 0.0                          # core 7: zero ghost0 h

    xf = x[:, :, :, ::-1]                             # W-flip for R direction
    in_maps = []
    for c in range(NCORES):
        xloc = np.zeros((4, CIN, H, W), np.float32)
        xfloc = np.zeros((4, CIN, H, W), np.float32)
        nb = min(4, B - 2 * c)
        xloc[:nb] = x[2 * c:2 * c + nb]
        xfloc[:nb] = xf[2 * c:2 * c + nb]
        dl = _diag_pack(xloc)                         # [WD, 2, 128, 256]
        dr = _diag_pack(xfloc)
        xd = np.stack([dl, dr], axis=3)               # [WD, 2, 128, 2, 256]
        in_maps.append({
            "xd": np.ascontiguousarray(xd),
            "wm": wm_t, "k0t": k0_t, "k1t": k1_t, "wih": wih_t,
            "misc": misc_all[c],
        })
    return in_maps


# ------------------------------------------------------- reference-free host
# numpy replica of the device program, for debugging (same per-core arrays)

def _core_sim(im, nsteps=WD):
    xd = im["xd"]                   # [WD, 2, 128, 2, 256]
    wm_t, k0_t, k1_t, wih_t = im["wm"], im["k0t"], im["k1t"], im["wih"]
    misc = im["misc"]

    def unT(t, nmt):                # tile array -> [out, in]
        return t.transpose(1, 0, 2, 3).reshape(CIN, nmt * 128).T

    wm, k0, k1, wih = unT(wm_t, 2), unT(k0_t, 2), unT(k1_t, 2), unT(wih_t, 8)
    bias_u = misc[:, 0:2].T.reshape(CIN)
    bias_g = misc[:, 2:10].T.reshape(8 * 128)
    s = misc[0, 10]

    def sig(v):
        return 1.0 / (1.0 + np.exp(-v))

    h = np.zeros((CIN, 2 * NCOL), np.float32)    # [ch, dir*col]
    hv = h.reshape(CIN, 2, NCOL)
    cst = np.zeros((CIN, 2 * NDATA), np.float32)
    out = np.zeros((WD, 2, 128, 2, OWN), np.float32)
    hp = np.empty((CIN, 2, NDATA), np.float32)
    hn = np.empty((CIN, 2, NDATA), np.float32)
    for d in range(nsteps):
        xs = np.ascontiguousarray(
            xd[d].reshape(CIN, 2, NCOL)[:, :, :NDATA]).reshape(CIN, -1)
        hp[:] = hv[:, :, 0:NDATA]
        hn[:] = hv[:, :, 1:NDATA + 1]
        u = (wm @ xs + k0 @ hp.reshape(CIN, -1) + k1 @ hn.reshape(CIN, -1)
             + bias_u[:, None])
        g = wih @ u + bias_g[:, None]
        gi, gf, gg, go = g[0:256], g[256:512], g[512:768], g[768:1024]
        cst = sig(gf) * cst + sig(gi) * np.tanh(gg)
        hv[:, :, 0:NDATA] = (sig(go) * np.tanh(cst)).reshape(CIN, 2, NDATA)
        hv[:, :, 128] *= s
        out[d] = hv.reshape(2, 128, 2, NCOL)[:, :, :, 0:OWN]
    return out


# ----------------------------------------------------------- output assembly

def _assemble(core_outs):
    # core_outs: list of [WD, 2, 128, 2, OWN] -> hs [2dir, WD, 256ch, 1024rows]
    hs = np.zeros((2, WD, CIN, B * H), np.float32)
    for c, o in enumerate(core_outs):
        o = np.asarray(o)
        hs[:, :, :, c * OWN:(c + 1) * OWN] = (
            o.transpose(3, 0, 1, 2, 4).reshape(2, WD, CIN, OWN))

    def unscramble(hd):             # [WD, 256ch, 1024rows] -> [B, COUT, H, WD]
        a = hd.transpose(0, 2, 1).reshape(WD, B, COUT, H)
        return a.transpose(1, 2, 3, 0)

    def unshift(a):                 # [B, COUT, H, WD] -> [B, COUT, H, W]
        rows = np.arange(H)[:, None]
        cols = rows + np.arange(W)[None, :]
        return a[:, :, rows, cols]

    left = unshift(unscramble(hs[0]))
    right = unshift(unscramble(hs[1]))[:, :, :, ::-1]
    right = np.concatenate(
        [np.zeros_like(right[:, :, :1, :]), right[:, :, :-1, :]], axis=2)
    return left + right


# --------------------------------------------------------------- bass kernel

def _build(nsteps=WD):
    import concourse.bacc as bacc
    import concourse.mybir as mybir
    import concourse.tile as tile
    from concourse._compat import get_trn_type

    f32 = mybir.dt.float32
    f32r = mybir.dt.float32r
    AF = mybir.ActivationFunctionType

    nc = bacc.Bacc(get_trn_type() or "TRN2", target_bir_lowering=False,
                   debug=False)
    xd = nc.dram_tensor("xd", [WD, NKC, 128, 2, NCOL], f32r,
                        kind="ExternalInput")
    wm = nc.dram_tensor("wm", [128, NKC, 2, 128], f32r, kind="ExternalInput")
    k0t = nc.dram_tensor("k0t", [128, NKC, 2, 128], f32r, kind="ExternalInput")
    k1t = nc.dram_tensor("k1t", [128, NKC, 2, 128], f32r, kind="ExternalInput")
    wih = nc.dram_tensor("wih", [128, NKC, NGT, 128], f32r,
                         kind="ExternalInput")
    misc = nc.dram_tensor("misc", [128, 12], f32, kind="ExternalInput")
    hs_out = nc.dram_tensor("hs", [WD, NKC, 128, 2, OWN], f32r,
                            kind="ExternalOutput")

    with tile.TileContext(nc) as tc:
        with (
            tc.tile_pool(name="wpool", bufs=1) as wpool,
            tc.tile_pool(name="state", bufs=1) as state,
            tc.tile_pool(name="xpool", bufs=3) as xpool,
            tc.tile_pool(name="upool", bufs=2) as upool,
            tc.tile_pool(name="apool", bufs=2) as apool,
            tc.tile_pool(name="tpool", bufs=2) as tpool,
            tc.tile_pool(name="upsum", bufs=2, space="PSUM") as upsum,
            tc.tile_pool(name="gpsum", bufs=5, space="PSUM") as gpsum,
        ):
            wm_t = wpool.tile([128, NKC, 2, 128], f32r, tag="wm")
            k0_t = wpool.tile([128, NKC, 2, 128], f32r, tag="k0")
            k1_t = wpool.tile([128, NKC, 2, 128], f32r, tag="k1")
            wih_t = wpool.tile([128, NKC, NGT, 128], f32r, tag="wih")
            misc_t = wpool.tile([128, 12], f32, tag="misc")
            nc.sync.dma_start(wm_t[:], wm[:])
            nc.sync.dma_start(k0_t[:], k0t[:])
            nc.sync.dma_start(k1_t[:], k1t[:])
            nc.sync.dma_start(wih_t[:], wih[:])
            nc.sync.dma_start(misc_t[:], misc[:])

            h = state.tile([128, NKC, 2, NCOL], f32r, tag="h")
            cs = state.tile([128, NKC, 2, NDATA], f32, tag="c")
            nc.any.memset(h[:].bitcast(f32), 0.0)
            nc.any.memset(cs[:], 0.0)

            for d in range(nsteps):
                xs = xpool.tile([128, NKC, 2, NCOL], f32r, tag="xs")
                for kc in range(NKC):
                    nc.sync.dma_start(xs[:, kc], xd[d, kc])

                u = upool.tile([128, NKC, 2, NDATA], f32r, tag="u")
                for m in range(NKC):
                    up = upsum.tile([128, 2, NDATA], f32, tag="up")
                    for kc in range(NKC):
                        nc.tensor.matmul(
                            up[:], wm_t[:, kc, m, :],
                            xs[:, kc, :, 0:NDATA],
                            start=(kc == 0), stop=False)
                    for kc in range(NKC):
                        nc.tensor.matmul(
                            up[:], k0_t[:, kc, m, :],
                            h[:, kc, :, 0:NDATA], start=False, stop=False)
                    for kc in range(NKC):
                        nc.tensor.matmul(
                            up[:], k1_t[:, kc, m, :],
                            h[:, kc, :, 1:NDATA + 1],
                            start=False, stop=(kc == NKC - 1))
                    nc.vector.tensor_scalar_add(
                        u[:, m], up[:], misc_t[:, m:m + 1])

                acts = []
                for t in range(NGT):
                    gp = gpsum.tile([128, 2, NDATA], f32, tag="gp")
                    for kc in range(NKC):
                        nc.tensor.matmul(
                            gp[:], wih_t[:, kc, t, :], u[:, kc],
                            start=(kc == 0), stop=(kc == NKC - 1))
                    a = apool.tile([128, 2, NDATA], f32, tag=f"act{t}")
                    fn = AF.Tanh if t in (4, 5) else AF.Sigmoid
                    nc.scalar.activation(a[:], gp[:], fn,
                                         bias=misc_t[:, 2 + t:3 + t])
                    acts.append(a)

                for m in range(NKC):
                    t1 = tpool.tile([128, 2, NDATA], f32, tag=f"t1_{m}")
                    nc.vector.tensor_mul(t1[:], acts[0 + m][:], acts[4 + m][:])
                    nc.vector.tensor_mul(cs[:, m], cs[:, m], acts[2 + m][:])
                    nc.vector.tensor_add(cs[:, m], cs[:, m], t1[:])
                    t2 = tpool.tile([128, 2, NDATA], f32, tag=f"t2_{m}")
                    nc.scalar.activation(t2[:], cs[:, m], AF.Tanh)
                    nc.vector.tensor_mul(h[:, m, :, 0:NDATA], acts[6 + m][:],
                                         t2[:])
                nc.vector.tensor_scalar_mul(
                    h[:, :, :, OWN:OWN + 1], h[:, :, :, OWN:OWN + 1],
                    misc_t[:, 10:11])

                for kc in range(NKC):
                    nc.sync.dma_start(hs_out[d, kc], h[:, kc, :, 0:OWN])

    nc.finalize()
    return nc


def _get_compiled(nsteps=WD):
    if nsteps not in _COMPILED:
        _COMPILED[nsteps] = _build(nsteps)
    return _COMPILED[nsteps]


# ------------------------------------------------------------------- driver

def kernel(x, w_i2s, b_i2s, w_ih, b_ih, b_hh, k0, k1, b_s2s):
    from concourse.bass_utils import run_bass_kernel_spmd

    in_maps = _prep_inputs(np.asarray(x, np.float32), np.asarray(w_i2s),
                           np.asarray(b_i2s), np.asarray(w_ih),
                           np.asarray(b_ih), np.asarray(b_hh),
                           np.asarray(k0), np.asarray(k1), np.asarray(b_s2s))
    nc = _get_compiled()
    res = run_bass_kernel_spmd(nc, in_maps, list(range(NCORES)))
    return _assemble([res.results[c]["hs"] for c in range(NCORES)])


def kernel_numpy(x, w_i2s, b_i2s, w_ih, b_ih, b_hh, k0, k1, b_s2s):
    """Host-only replica of the device program (debug path)."""
    in_maps = _prep_inputs(np.asarray(x, np.float32), np.asarray(w_i2s),
                           np.asarray(b_i2s), np.asarray(w_ih),
                           np.asarray(b_ih), np.asarray(b_hh),
                           np.asarray(k0), np.asarray(k1), np.asarray(b_s2s))
    return _assemble([_core_sim(im) for im in in_maps])



# revision 8
# speedup vs baseline: 1.4805x; 1.4805x over previous
"""DiagonalBiLSTM Trainium2 kernel, v2.

Full inputs in, full output out. Sharding: direction-parallel x row-parallel.
Cores 0-3 run the left-to-right diagonal scan over row blocks of 256
(4 batches each); cores 4-7 run the right-to-left scan (x W-flipped).
Each core computes 256 own rows plus a shrinking ghost region (126-d extra
rows at diagonal step d) so no inter-core communication is needed.

Per diagonal step d (n = 382 - d active columns, 3 column chunks <=128):
    u  = xs[d] + k0 @ h + k1 @ h(+1)        (xs = wm@x + b_i2s + b_s2s,
                                             precomputed on host, fp16)
    g  = w_ih @ u + bias                    (bias via K-hot bias matmuls)
    c  = sig(g_f) * c + sig(g_i) * tanh(g_g)
    h  = sig(g_o) * tanh(c)

All matmul operands fp16 (full-rate at any free dim, LDWEIGHTS hidden);
c state fp32. Column chunks are software-pipelined so the PE never idles
(avoids HAM clock-gate re-throttling) and Act/DVE run concurrently.
"""

import numpy as np

B, CIN, COUT, H, W, DC = 16, 256, 256, 64, 64, 3
WD = 2 * W - 1          # 127 diagonal steps
NCORES = 8
NDIRCORES = 4           # cores per direction
OWN = 256               # own rows per core (4 batches)
GHOST = 126
NDATA = OWN + GHOST     # 382 max active columns
NPAD = 384
NKC = 2                 # channel chunks (256 = 2*128)
NGT = 8                 # gate m-tiles (1024 = 8*128)
NC_MAX = 128            # max chunk width

# gate slot order in PSUM: [i0,i1,f0,f1,o0,o1,g0,g1] (w_ih row blocks)
SLOT_ROWS = [(0, 128), (128, 256), (256, 384), (384, 512),
             (768, 896), (896, 1024), (512, 640), (640, 768)]

_COMPILED = {}


# ----------------------------------------------------------------- host prep

def _i2s_mask_np():
    oc = np.arange(COUT) % DC
    ic = np.arange(CIN) % DC
    return (ic[None, :] <= oc[:, None]).astype(np.float32)


def _wT_tiles(w, nmt):
    # [out=nmt*128, in=256] -> lhsT tile array [k=128, kc=2, mt=nmt, m=128]
    return np.ascontiguousarray(
        w.T.reshape(NKC, 128, nmt, 128).transpose(1, 0, 2, 3))


def _prep_inputs(x, w_i2s, b_i2s, w_ih, b_ih, b_hh, k0, k1, b_s2s):
    x = np.asarray(x, np.float32)
    wm = (np.asarray(w_i2s, np.float32) * _i2s_mask_np())
    bias_u = (np.asarray(b_i2s, np.float32)
              + np.asarray(b_s2s, np.float32))          # [256]
    bias_g = (np.asarray(b_ih, np.float32)
              + np.asarray(b_hh, np.float32))           # [1024]

    k0_t = _wT_tiles(np.asarray(k0, np.float32), 2).astype(np.float16)
    k1_t = _wT_tiles(np.asarray(k1, np.float32), 2).astype(np.float16)

    # gate weight tiles in slot order: wih[k, kc, t, m]
    wih = np.empty((128, NKC, NGT, 128), np.float32)
    for t, (r0, r1) in enumerate(SLOT_ROWS):
        wt = _wT_tiles(np.asarray(w_ih, np.float32)[r0:r1], 1)  # [128,2,1,128]
        wih[:, :, t, :] = wt[:, :, 0, :]
    wih = wih.astype(np.float16)

    # bias stationaries: bgw[k, t, m] = bias_g[slot t, ch m] if k == 0
    bgw = np.zeros((128, NGT, 128), np.float32)
    for t, (r0, r1) in enumerate(SLOT_ROWS):
        bgw[0, t, :] = bias_g[r0:r1]
    bgw = bgw.astype(np.float16)

    # xs = wm @ x + bias_u for both directions: [B, 256, H, W]
    x2 = np.ascontiguousarray(x.transpose(1, 0, 2, 3)).reshape(CIN, -1)
    xs_l = np.ascontiguousarray(
        (wm @ x2).reshape(COUT, B, H, W).transpose(1, 0, 2, 3))
    xs_l += bias_u[None, :, None, None]
    xs_r = xs_l[:, :, :, ::-1]

    in_maps = []
    for core in range(NCORES):
        xs_d = xs_l if core < NDIRCORES else xs_r
        c0 = (core % NDIRCORES) * 4                     # first batch
        # rows r = 0..381 -> (b, h) = ((c0*64*4 + r)//64, r%64); rows beyond
        # B*H are bias-only (nonexistent ghost rows on the last core).
        xd = np.empty((WD, CIN, NPAD), np.float32)
        xd[:] = bias_u[None, :, None]                   # out-of-band fill
        nrows = min(NDATA, B * H - c0 * H)              # 382 or 256
        rb = np.arange(nrows)
        bs, hs = c0 + rb // H, rb % H
        # xd[d, :, r] = xs_d[b, :, h, d - h] when 0 <= d-h < W
        for h in range(H):
            sel = hs == h
            if not sel.any():
                continue
            rows = rb[sel]
            # steps d = h..h+W-1 map to w = 0..W-1
            blk = xs_d[bs[sel], :, h, :]                # [nr, 256, W]
            xd[h:h + W][:, :, rows] = blk.transpose(2, 1, 0)
        in_maps.append({
            "xd": np.ascontiguousarray(xd.reshape(WD, NKC, 128, NPAD)
                                       ).astype(np.float16),
            "k0t": k0_t, "k1t": k1_t, "wih": wih, "bgw": bgw,
            "misc": np.full((128, 4),
                            0.0 if core % NDIRCORES == NDIRCORES - 1 else 1.0,
                            np.float32),
        })
    return in_maps


# ----------------------------------------------------------- output assembly

def _assemble(core_outs):
    # core_outs: list of [WD, 2, 128, OWN] -> hs [2dir, WD, 256ch, 1024rows]
    hs = np.zeros((2, WD, CIN, B * H), np.float32)
    for c, o in enumerate(core_outs):
        d = c // NDIRCORES
        j = (c % NDIRCORES) * OWN
        hs[d, :, :, j:j + OWN] = np.asarray(o, np.float32).reshape(
            WD, CIN, OWN)

    def unscramble(hd):             # [WD, 256ch, 1024rows] -> [B, COUT, H, WD]
        a = hd.transpose(0, 2, 1).reshape(WD, B, COUT, H)
        return a.transpose(1, 2, 3, 0)

    def unshift(a):                 # [B, COUT, H, WD] -> [B, COUT, H, W]
        rows = np.arange(H)[:, None]
        cols = rows + np.arange(W)[None, :]
        return a[:, :, rows, cols]

    left = unshift(unscramble(hs[0]))
    right = unshift(unscramble(hs[1]))[:, :, :, ::-1]
    right = np.concatenate(
        [np.zeros_like(right[:, :, :1, :]), right[:, :, :-1, :]], axis=2)
    return left + right


# ------------------------------------------------------- reference-free host
# numpy replica of the device program (fp16 rounding modeled), for debugging

def _core_sim(im, nsteps=WD):
    f16 = np.float16
    xd = im["xd"]                        # [WD, 2, 128, NPAD] fp16

    def unT(t, nmt):                     # [k,kc,mt,m] -> [out, in] f32
        return np.float32(t).transpose(1, 0, 2, 3).reshape(
            CIN, nmt * 128).T

    k0 = unT(im["k0t"], 2)
    k1 = unT(im["k1t"], 2)
    wih_t = np.float32(im["wih"])        # [k, kc, t, m]
    bg = np.float32(im["bgw"])[0]        # [t, m]
    scale = im["misc"][0, 0]

    def sig(v):
        return 1.0 / (1.0 + np.exp(-v))

    h = np.zeros((CIN, NPAD), np.float32)     # ch x col, col 382+ stays 0
    c = np.zeros((CIN, NPAD), np.float32)
    out = np.zeros((nsteps, CIN, OWN), f16)
    for d in range(nsteps):
        n = NDATA - d
        xs = np.float32(xd[d].reshape(CIN, NPAD))[:, :n]
        u = f16(xs + k0 @ h[:, :n] + k1 @ h[:, 1:n + 1]).astype(np.float32)
        # gates per slot
        g = np.empty((NGT, 128, n), np.float32)
        for t in range(NGT):
            acc = bg[t][:, None] * np.ones((1, n), np.float32)
            for kc in range(NKC):
                acc = acc + wih_t[:, kc, t].T @ u[kc * 128:(kc + 1) * 128]
            g[t] = acc
        gi = np.concatenate([g[0], g[1]])
        gf = np.concatenate([g[2], g[3]])
        go = np.concatenate([g[4], g[5]])
        gg = np.concatenate([g[6], g[7]])
        t1 = f16(f16(sig(gi)) * f16(np.tanh(gg))).astype(np.float32)
        c[:, :n] = c[:, :n] * f16(sig(gf)).astype(np.float32) + t1
        h[:, :n] = f16(f16(sig(go)) * f16(np.tanh(c[:, :n]))).astype(
            np.float32)
        if d < WD - 1:
            h[:, OWN] *= scale
        out[d] = f16(h[:, :OWN])
    return out


# --------------------------------------------------------------- bass kernel

def _build(nsteps=WD):
    import concourse.bacc as bacc
    import concourse.mybir as mybir
    import concourse.tile as tile
    from concourse._compat import get_trn_type

    f32 = mybir.dt.float32
    f16 = mybir.dt.float16
    AF = mybir.ActivationFunctionType

    nc = bacc.Bacc(get_trn_type() or "TRN2", target_bir_lowering=False,
                   debug=False)
    xd = nc.dram_tensor("xd", [WD, NKC, 128, NPAD], f16, kind="ExternalInput")
    k0t = nc.dram_tensor("k0t", [128, NKC, 2, 128], f16, kind="ExternalInput")
    k1t = nc.dram_tensor("k1t", [128, NKC, 2, 128], f16, kind="ExternalInput")
    wih = nc.dram_tensor("wih", [128, NKC, NGT, 128], f16,
                         kind="ExternalInput")
    bgw = nc.dram_tensor("bgw", [128, NGT, 128], f16, kind="ExternalInput")
    misc = nc.dram_tensor("misc", [128, 4], f32, kind="ExternalInput")
    hs_out = nc.dram_tensor("hs", [WD, NKC, 128, OWN], f16,
                            kind="ExternalOutput")

    with tile.TileContext(nc) as tc:
        with (
            tc.tile_pool(name="wpool", bufs=1) as wpool,
            tc.tile_pool(name="state", bufs=1) as state,
            tc.tile_pool(name="xpool", bufs=4) as xpool,
            tc.tile_pool(name="upool", bufs=3) as upool,
            tc.tile_pool(name="apool", bufs=3) as apool,
            tc.tile_pool(name="upsum", bufs=3, space="PSUM") as upsum,
            tc.tile_pool(name="gpsum", bufs=2, space="PSUM") as gpsum,
        ):
            k0_t = wpool.tile([128, NKC, 2, 128], f16, tag="k0")
            k1_t = wpool.tile([128, NKC, 2, 128], f16, tag="k1")
            wih_t = wpool.tile([128, NKC, NGT, 128], f16, tag="wih")
            bgw_t = wpool.tile([128, NGT, 128], f16, tag="bgw")
            misc_t = wpool.tile([128, 4], f32, tag="misc")
            ones_t = wpool.tile([128, NC_MAX], f16, tag="ones")
            nc.sync.dma_start(k0_t[:], k0t[:])
            nc.sync.dma_start(k1_t[:], k1t[:])
            nc.sync.dma_start(wih_t[:], wih[:])
            nc.sync.dma_start(bgw_t[:], bgw[:])
            nc.sync.dma_start(misc_t[:], misc[:])
            nc.any.memset(ones_t[:], 1.0)

            h = state.tile([128, NKC, NPAD], f16, tag="h")
            cs = state.tile([128, NKC, NPAD], f32, tag="c")
            nc.any.memset(h[:], 0.0)
            nc.any.memset(cs[:], 0.0)

            PF = 2      # xs prefetch depth
            xs_tiles = {}
            for dd in range(min(PF + 1, nsteps)):
                t = xpool.tile([128, NKC, NPAD], f16, tag="xs", name=f"xs_pf")
                for kc in range(NKC):
                    nc.sync.dma_start(t[:, kc], xd[dd, kc])
                xs_tiles[dd] = t

            def emit_umm(uP, lo, hi):
                for m in range(NKC):
                    nc.tensor.matmul(uP[:, m, 0:hi - lo],
                                     k0_t[:, 0, m, :], h[:, 0, lo:hi],
                                     start=True, stop=False)
                    nc.tensor.matmul(uP[:, m, 0:hi - lo],
                                     k0_t[:, 1, m, :], h[:, 1, lo:hi],
                                     start=False, stop=False)
                    nc.tensor.matmul(uP[:, m, 0:hi - lo],
                                     k1_t[:, 0, m, :], h[:, 0, lo + 1:hi + 1],
                                     start=False, stop=False)
                    nc.tensor.matmul(uP[:, m, 0:hi - lo],
                                     k1_t[:, 1, m, :], h[:, 1, lo + 1:hi + 1],
                                     start=False, stop=True)

            def emit_gates(gP, u_sb, n):
                for t in range(NGT):
                    nc.tensor.matmul(gP[:, t, 0:n], bgw_t[:, t, :],
                                     ones_t[:, 0:n], start=True, stop=False)
                    nc.tensor.matmul(gP[:, t, 0:n], wih_t[:, 0, t, :],
                                     u_sb[:, 0, 0:n], start=False, stop=False)
                    nc.tensor.matmul(gP[:, t, 0:n], wih_t[:, 1, t, :],
                                     u_sb[:, 1, 0:n], start=False, stop=True)

            def emit_uadd(u_sb, uP, xs_t, lo, hi):
                nc.vector.tensor_add(u_sb[:, :, 0:hi - lo],
                                     uP[:, :, 0:hi - lo], xs_t[:, :, lo:hi])

            def emit_acts(gP, ifo, gt, n):
                nc.scalar.activation(ifo[:, :, 0:n], gP[:, 0:6, 0:n],
                                     AF.Sigmoid)
                nc.scalar.activation(gt[:, :, 0:n], gP[:, 6:8, 0:n], AF.Tanh)

            def emit_cell1(ifo, gt, t1, lo, hi):
                n = hi - lo
                nc.vector.tensor_mul(t1[:, :, 0:n], ifo[:, 0:2, 0:n],
                                     gt[:, :, 0:n])
                nc.vector.tensor_mul(cs[:, :, lo:hi], cs[:, :, lo:hi],
                                     ifo[:, 2:4, 0:n])
                nc.vector.tensor_add(cs[:, :, lo:hi], cs[:, :, lo:hi],
                                     t1[:, :, 0:n])

            def emit_cell2(ifo, tc_t, lo, hi):
                n = hi - lo
                nc.vector.tensor_mul(h[:, :, lo:hi], ifo[:, 4:6, 0:n],
                                     tc_t[:, :, 0:n])

            for d in range(nsteps):
                n = NDATA - d
                s1, s2 = 126 - d, 254 - d
                chunks = [(0, s1), (s1, s2), (s2, n)]
                chunks = [(lo, hi) for lo, hi in chunks if hi > lo]
                xs_t = xs_tiles.pop(d)
                if d + PF + 1 < nsteps:
                    t = xpool.tile([128, NKC, NPAD], f16, tag="xs", name=f"xs_pf")
                    for kc in range(NKC):
                        nc.sync.dma_start(t[:, kc], xd[d + PF + 1, kc])
                    xs_tiles[d + PF + 1] = t

                uPs, usbs, gPs, ifos, gts = {}, {}, {}, {}, {}

                def chunk_u(i):
                    lo, hi = chunks[i]
                    uPs[i] = upsum.tile([128, NKC, NC_MAX], f32, tag="uP", name=f"uP{d}_{i}")
                    emit_umm(uPs[i], lo, hi)

                def chunk_uadd(i):
                    lo, hi = chunks[i]
                    usbs[i] = upool.tile([128, NKC, NC_MAX], f16, tag="usb", name=f"usb{d}_{i}")
                    emit_uadd(usbs[i], uPs[i], xs_t, lo, hi)

                def chunk_gates(i):
                    lo, hi = chunks[i]
                    gPs[i] = gpsum.tile([128, NGT, NC_MAX], f32, tag="gP", name=f"gP{d}_{i}")
                    emit_gates(gPs[i], usbs[i], hi - lo)

                def chunk_acts(i):
                    lo, hi = chunks[i]
                    ifos[i] = apool.tile([128, 6, NC_MAX], f16, tag="ifo", name=f"ifo{d}_{i}")
                    gts[i] = apool.tile([128, 2, NC_MAX], f16, tag="gt", name=f"gt{d}_{i}")
                    emit_acts(gPs[i], ifos[i], gts[i], hi - lo)

                def chunk_cell1(i):
                    lo, hi = chunks[i]
                    t1 = apool.tile([128, 2, NC_MAX], f16, tag="t1", name=f"t1_{d}_{i}")
                    emit_cell1(ifos[i], gts[i], t1, lo, hi)

                def chunk_tanhc(i):
                    lo, hi = chunks[i]
                    tc_t = apool.tile([128, 2, NC_MAX], f16, tag="tc", name=f"tc{d}_{i}")
                    nc.scalar.activation(tc_t[:, :, 0:hi - lo],
                                         cs[:, :, lo:hi], AF.Tanh)
                    return tc_t

                def chunk_cell2(i, tc_t):
                    lo, hi = chunks[i]
                    emit_cell2(ifos[i], tc_t, lo, hi)

                nch = len(chunks)
                if nch == 3:
                    chunk_u(0)
                    chunk_uadd(0)
                    chunk_gates(0)
                    chunk_u(1)
                    chunk_uadd(1)
                    chunk_acts(0)
                    chunk_cell1(0)
                    tc0 = chunk_tanhc(0)
                    chunk_cell2(0, tc0)
                    chunk_u(2)
                    chunk_uadd(2)
                    chunk_gates(1)
                    chunk_acts(1)
                    chunk_cell1(1)
                    tc1 = chunk_tanhc(1)
                    chunk_cell2(1, tc1)
                    chunk_gates(2)
                    chunk_acts(2)
                    chunk_cell1(2)
                    tc2 = chunk_tanhc(2)
                    chunk_cell2(2, tc2)
                else:
                    for i in range(nch):
                        chunk_u(i)
                        chunk_uadd(i)
                        chunk_gates(i)
                        chunk_acts(i)
                        chunk_cell1(i)
                        tci = chunk_tanhc(i)
                        chunk_cell2(i, tci)

                if d < nsteps - 1:
                    nc.gpsimd.tensor_scalar_mul(
                        h[:, :, OWN:OWN + 1], h[:, :, OWN:OWN + 1],
                        misc_t[:, 0:1])

                for kc in range(NKC):
                    nc.sync.dma_start(hs_out[d, kc], h[:, kc, 0:OWN])

    nc.finalize()
    return nc


def _get_compiled(nsteps=WD):
    if nsteps not in _COMPILED:
        _COMPILED[nsteps] = _build(nsteps)
    return _COMPILED[nsteps]


# ------------------------------------------------------------------- driver

def kernel(x, w_i2s, b_i2s, w_ih, b_ih, b_hh, k0, k1, b_s2s):
    from concourse.bass_utils import run_bass_kernel_spmd

    in_maps = _prep_inputs(x, w_i2s, b_i2s, w_ih, b_ih, b_hh, k0, k1, b_s2s)
    nc = _get_compiled()
    res = run_bass_kernel_spmd(nc, in_maps, list(range(NCORES)))
    return _assemble([np.asarray(res.results[c]["hs"]).reshape(WD, CIN, OWN)
                      for c in range(NCORES)])


def kernel_numpy(x, w_i2s, b_i2s, w_ih, b_ih, b_hh, k0, k1, b_s2s):
    """Host-only replica of the device program (debug path)."""
    in_maps = _prep_inputs(x, w_i2s, b_i2s, w_ih, b_ih, b_hh, k0, k1, b_s2s)
    return _assemble([_core_sim(im).reshape(WD, CIN, OWN) for im in in_maps])


# revision 10
# speedup vs baseline: 1.7308x; 1.1691x over previous
"""DiagonalBiLSTM Trainium2 kernel, v2.

Full inputs in, full output out. Sharding: direction-parallel x row-parallel.
Cores 0-3 run the left-to-right diagonal scan over row blocks of 256
(4 batches each); cores 4-7 run the right-to-left scan (x W-flipped).
Each core computes 256 own rows plus a shrinking ghost region (126-d extra
rows at diagonal step d) so no inter-core communication is needed.

Per diagonal step d (n = 382 - d active columns, 3 column chunks <=128):
    u  = xs[d] + k0 @ h + k1 @ h(+1)        (xs = wm@x + b_i2s + b_s2s,
                                             precomputed on host, fp16)
    g  = w_ih @ u + bias                    (bias via K-hot bias matmuls)
    c  = sig(g_f) * c + sig(g_i) * tanh(g_g)
    h  = sig(g_o) * tanh(c)

All matmul operands fp16 (full-rate at any free dim, LDWEIGHTS hidden);
c state fp32. Column chunks are software-pipelined so the PE never idles
(avoids HAM clock-gate re-throttling) and Act/DVE run concurrently.
"""

import numpy as np

B, CIN, COUT, H, W, DC = 16, 256, 256, 64, 64, 3
WD = 2 * W - 1          # 127 diagonal steps
NCORES = 8
NDIRCORES = 4           # cores per direction
OWN = 256               # own rows per core (4 batches)
GHOST = 126
NDATA = OWN + GHOST     # 382 max active columns
NPAD = 384
NKC = 2                 # channel chunks (256 = 2*128)
NGT = 8                 # gate m-tiles (1024 = 8*128)
NC_MAX = 128            # max chunk width

# gate slot order in PSUM: [i0,i1,f0,f1,o0,o1,g0,g1] (w_ih row blocks)
SLOT_ROWS = [(0, 128), (128, 256), (256, 384), (384, 512),
             (768, 896), (896, 1024), (512, 640), (640, 768)]

_COMPILED = {}


# ----------------------------------------------------------------- host prep

def _i2s_mask_np():
    oc = np.arange(COUT) % DC
    ic = np.arange(CIN) % DC
    return (ic[None, :] <= oc[:, None]).astype(np.float32)


def _wT_tiles(w, nmt):
    # [out=nmt*128, in=256] -> lhsT tile array [k=128, kc=2, mt=nmt, m=128]
    return np.ascontiguousarray(
        w.T.reshape(NKC, 128, nmt, 128).transpose(1, 0, 2, 3))


def _prep_inputs(x, w_i2s, b_i2s, w_ih, b_ih, b_hh, k0, k1, b_s2s):
    x = np.asarray(x, np.float32)
    wm = (np.asarray(w_i2s, np.float32) * _i2s_mask_np())
    bias_u = (np.asarray(b_i2s, np.float32)
              + np.asarray(b_s2s, np.float32))          # [256]
    bias_g = (np.asarray(b_ih, np.float32)
              + np.asarray(b_hh, np.float32))           # [1024]

    k0_t = _wT_tiles(np.asarray(k0, np.float32), 2).astype(np.float16)
    k1_t = _wT_tiles(np.asarray(k1, np.float32), 2).astype(np.float16)

    # gate weight tiles in slot order: wih[k, kc, t, m]
    wih = np.empty((128, NKC, NGT, 128), np.float32)
    for t, (r0, r1) in enumerate(SLOT_ROWS):
        wt = _wT_tiles(np.asarray(w_ih, np.float32)[r0:r1], 1)  # [128,2,1,128]
        wih[:, :, t, :] = wt[:, :, 0, :]
    wih = wih.astype(np.float16)

    # bias stationaries: bgw[k, t, m] = bias_g[slot t, ch m] if k == 0
    bgw = np.zeros((128, NGT, 128), np.float32)
    for t, (r0, r1) in enumerate(SLOT_ROWS):
        bgw[0, t, :] = bias_g[r0:r1]
    bgw = bgw.astype(np.float16)

    # xs = wm @ x + bias_u for both directions: [B, 256, H, W]
    x2 = np.ascontiguousarray(x.transpose(1, 0, 2, 3)).reshape(CIN, -1)
    xs_l = np.ascontiguousarray(
        (wm @ x2).reshape(COUT, B, H, W).transpose(1, 0, 2, 3))
    xs_l += bias_u[None, :, None, None]
    xs_r = xs_l[:, :, :, ::-1]

    in_maps = []
    for core in range(NCORES):
        xs_d = xs_l if core < NDIRCORES else xs_r
        c0 = (core % NDIRCORES) * 4                     # first batch
        # rows r = 0..381 -> (b, h) = ((c0*64*4 + r)//64, r%64); rows beyond
        # B*H are bias-only (nonexistent ghost rows on the last core).
        xd = np.empty((WD, CIN, NPAD), np.float32)
        xd[:] = bias_u[None, :, None]                   # out-of-band fill
        nrows = min(NDATA, B * H - c0 * H)              # 382 or 256
        rb = np.arange(nrows)
        bs, hs = c0 + rb // H, rb % H
        # xd[d, :, r] = xs_d[b, :, h, d - h] when 0 <= d-h < W
        for h in range(H):
            sel = hs == h
            if not sel.any():
                continue
            rows = rb[sel]
            # steps d = h..h+W-1 map to w = 0..W-1
            blk = xs_d[bs[sel], :, h, :]                # [nr, 256, W]
            xd[h:h + W][:, :, rows] = blk.transpose(2, 1, 0)
        in_maps.append({
            "xd": np.ascontiguousarray(xd.reshape(WD, NKC, 128, NPAD)
                                       ).astype(np.float16),
            "k0t": k0_t, "k1t": k1_t, "wih": wih, "bgw": bgw,
            "misc": np.full((128, 4),
                            0.0 if core % NDIRCORES == NDIRCORES - 1 else 1.0,
                            np.float32),
        })
    return in_maps


# ----------------------------------------------------------- output assembly

def _assemble(core_outs):
    # core_outs: list of [WD, 2, 128, OWN] -> hs [2dir, WD, 256ch, 1024rows]
    hs = np.zeros((2, WD, CIN, B * H), np.float32)
    for c, o in enumerate(core_outs):
        d = c // NDIRCORES
        j = (c % NDIRCORES) * OWN
        hs[d, :, :, j:j + OWN] = np.asarray(o, np.float32).reshape(
            WD, CIN, OWN)

    def unscramble(hd):             # [WD, 256ch, 1024rows] -> [B, COUT, H, WD]
        a = hd.transpose(0, 2, 1).reshape(WD, B, COUT, H)
        return a.transpose(1, 2, 3, 0)

    def unshift(a):                 # [B, COUT, H, WD] -> [B, COUT, H, W]
        rows = np.arange(H)[:, None]
        cols = rows + np.arange(W)[None, :]
        return a[:, :, rows, cols]

    left = unshift(unscramble(hs[0]))
    right = unshift(unscramble(hs[1]))[:, :, :, ::-1]
    right = np.concatenate(
        [np.zeros_like(right[:, :, :1, :]), right[:, :, :-1, :]], axis=2)
    return left + right


# ------------------------------------------------------- reference-free host
# numpy replica of the device program (fp16 rounding modeled), for debugging

def _core_sim(im, nsteps=WD):
    f16 = np.float16
    xd = im["xd"]                        # [WD, 2, 128, NPAD] fp16

    def unT(t, nmt):                     # [k,kc,mt,m] -> [out, in] f32
        return np.float32(t).transpose(1, 0, 2, 3).reshape(
            CIN, nmt * 128).T

    k0 = unT(im["k0t"], 2)
    k1 = unT(im["k1t"], 2)
    wih_t = np.float32(im["wih"])        # [k, kc, t, m]
    bg = np.float32(im["bgw"])[0]        # [t, m]
    scale = im["misc"][0, 0]

    def sig(v):
        return 1.0 / (1.0 + np.exp(-v))

    h = np.zeros((CIN, NPAD), np.float32)     # ch x col, col 382+ stays 0
    c = np.zeros((CIN, NPAD), np.float32)
    out = np.zeros((nsteps, CIN, OWN), f16)
    for d in range(nsteps):
        n = NDATA - d
        xs = np.float32(xd[d].reshape(CIN, NPAD))[:, :n]
        u = f16(xs + k0 @ h[:, :n] + k1 @ h[:, 1:n + 1]).astype(np.float32)
        # gates per slot
        g = np.empty((NGT, 128, n), np.float32)
        for t in range(NGT):
            acc = bg[t][:, None] * np.ones((1, n), np.float32)
            for kc in range(NKC):
                acc = acc + wih_t[:, kc, t].T @ u[kc * 128:(kc + 1) * 128]
            g[t] = acc
        gi = np.concatenate([g[0], g[1]])
        gf = np.concatenate([g[2], g[3]])
        go = np.concatenate([g[4], g[5]])
        gg = np.concatenate([g[6], g[7]])
        t1 = f16(f16(sig(gi)) * f16(np.tanh(gg))).astype(np.float32)
        c[:, :n] = c[:, :n] * f16(sig(gf)).astype(np.float32) + t1
        h[:, :n] = f16(f16(sig(go)) * f16(np.tanh(c[:, :n]))).astype(
            np.float32)
        if d < WD - 1:
            h[:, OWN] *= scale
        out[d] = f16(h[:, :OWN])
    return out


# --------------------------------------------------------------- bass kernel

def _build(nsteps=WD):
    import concourse.bacc as bacc
    import concourse.mybir as mybir
    import concourse.tile as tile
    from concourse._compat import get_trn_type

    f32 = mybir.dt.float32
    f16 = mybir.dt.float16
    AF = mybir.ActivationFunctionType

    nc = bacc.Bacc(get_trn_type() or "TRN2", target_bir_lowering=False,
                   debug=False)
    xd = nc.dram_tensor("xd", [WD, NKC, 128, NPAD], f16, kind="ExternalInput")
    k0t = nc.dram_tensor("k0t", [128, NKC, 2, 128], f16, kind="ExternalInput")
    k1t = nc.dram_tensor("k1t", [128, NKC, 2, 128], f16, kind="ExternalInput")
    wih = nc.dram_tensor("wih", [128, NKC, NGT, 128], f16,
                         kind="ExternalInput")
    bgw = nc.dram_tensor("bgw", [128, NGT, 128], f16, kind="ExternalInput")
    misc = nc.dram_tensor("misc", [128, 4], f32, kind="ExternalInput")
    hs_out = nc.dram_tensor("hs", [WD, NKC, 128, OWN], f16,
                            kind="ExternalOutput")

    with tile.TileContext(nc) as tc:
        with (
            tc.tile_pool(name="wpool", bufs=1) as wpool,
            tc.tile_pool(name="state", bufs=1) as state,
            tc.tile_pool(name="xpool", bufs=4) as xpool,
            tc.tile_pool(name="upool", bufs=4) as upool,
            tc.tile_pool(name="apool", bufs=4) as apool,
            tc.tile_pool(name="upsum", bufs=3, space="PSUM") as upsum,
            tc.tile_pool(name="gpsum", bufs=2, space="PSUM") as gpsum,
        ):
            k0_t = wpool.tile([128, NKC, 2, 128], f16, tag="k0")
            k1_t = wpool.tile([128, NKC, 2, 128], f16, tag="k1")
            wih_t = wpool.tile([128, NKC, NGT, 128], f16, tag="wih")
            bgw_t = wpool.tile([128, NGT, 128], f16, tag="bgw")
            misc_t = wpool.tile([128, 4], f32, tag="misc")
            ones_t = wpool.tile([128, NC_MAX], f16, tag="ones")
            nc.sync.dma_start(k0_t[:], k0t[:])
            nc.sync.dma_start(k1_t[:], k1t[:])
            nc.sync.dma_start(wih_t[:], wih[:])
            nc.sync.dma_start(bgw_t[:], bgw[:])
            nc.sync.dma_start(misc_t[:], misc[:])
            nc.any.memset(ones_t[:], 1.0)

            h = state.tile([128, NKC, NPAD], f16, tag="h")
            cs = state.tile([128, NKC, NPAD], f32, tag="c")
            nc.any.memset(h[:], 0.0)
            nc.any.memset(cs[:], 0.0)

            PF = 2      # xs prefetch depth
            xs_tiles = {}
            for dd in range(min(PF + 1, nsteps)):
                t = xpool.tile([128, NKC, NPAD], f16, tag="xs", name=f"xs_pf")
                for kc in range(NKC):
                    nc.sync.dma_start(t[:, kc], xd[dd, kc])
                xs_tiles[dd] = t

            def emit_umm(uP, lo, hi):
                for m in range(NKC):
                    nc.tensor.matmul(uP[:, m, 0:hi - lo],
                                     k0_t[:, 0, m, :], h[:, 0, lo:hi],
                                     start=True, stop=False)
                    nc.tensor.matmul(uP[:, m, 0:hi - lo],
                                     k0_t[:, 1, m, :], h[:, 1, lo:hi],
                                     start=False, stop=False)
                    nc.tensor.matmul(uP[:, m, 0:hi - lo],
                                     k1_t[:, 0, m, :], h[:, 0, lo + 1:hi + 1],
                                     start=False, stop=False)
                    nc.tensor.matmul(uP[:, m, 0:hi - lo],
                                     k1_t[:, 1, m, :], h[:, 1, lo + 1:hi + 1],
                                     start=False, stop=True)

            def emit_gates(gP, u_sb, n):
                for t in range(NGT):
                    nc.tensor.matmul(gP[:, t, 0:n], bgw_t[:, t, :],
                                     ones_t[:, 0:n], start=True, stop=False)
                    nc.tensor.matmul(gP[:, t, 0:n], wih_t[:, 0, t, :],
                                     u_sb[:, 0, 0:n], start=False, stop=False)
                    nc.tensor.matmul(gP[:, t, 0:n], wih_t[:, 1, t, :],
                                     u_sb[:, 1, 0:n], start=False, stop=True)

            def emit_uadd(u_sb, uP, xs_t, lo, hi):
                nc.vector.tensor_add(u_sb[:, :, 0:hi - lo],
                                     uP[:, :, 0:hi - lo], xs_t[:, :, lo:hi])

            def emit_acts(gP, ifo, gt, n):
                nc.scalar.activation(ifo[:, :, 0:n], gP[:, 0:6, 0:n],
                                     AF.Sigmoid)
                nc.scalar.activation(gt[:, :, 0:n], gP[:, 6:8, 0:n], AF.Tanh)

            def emit_cell1(ifo, gt, t1, lo, hi):
                n = hi - lo
                nc.vector.tensor_mul(t1[:, :, 0:n], ifo[:, 0:2, 0:n],
                                     gt[:, :, 0:n])
                nc.vector.tensor_mul(cs[:, :, lo:hi], cs[:, :, lo:hi],
                                     ifo[:, 2:4, 0:n])
                nc.vector.tensor_add(cs[:, :, lo:hi], cs[:, :, lo:hi],
                                     t1[:, :, 0:n])

            def emit_cell2(ifo, tc_t, lo, hi):
                n = hi - lo
                nc.vector.tensor_mul(h[:, :, lo:hi], ifo[:, 4:6, 0:n],
                                     tc_t[:, :, 0:n])

            for d in range(nsteps):
                n = NDATA - d
                s1, s2 = 126 - d, 254 - d
                chunks = [(0, s1), (s1, s2), (s2, n)]
                chunks = [(lo, hi) for lo, hi in chunks if hi > lo]
                xs_t = xs_tiles.pop(d)
                if d + PF + 1 < nsteps:
                    t = xpool.tile([128, NKC, NPAD], f16, tag="xs", name=f"xs_pf")
                    for kc in range(NKC):
                        nc.sync.dma_start(t[:, kc], xd[d + PF + 1, kc])
                    xs_tiles[d + PF + 1] = t

                uPs, usbs, gPs, ifos, gts = {}, {}, {}, {}, {}

                def chunk_u(i):
                    lo, hi = chunks[i]
                    uPs[i] = upsum.tile([128, NKC, NC_MAX], f32, tag="uP", name=f"uP{d}_{i}")
                    emit_umm(uPs[i], lo, hi)

                def chunk_uadd(i):
                    lo, hi = chunks[i]
                    usbs[i] = upool.tile([128, NKC, NC_MAX], f16, tag="usb", name=f"usb{d}_{i}")
                    emit_uadd(usbs[i], uPs[i], xs_t, lo, hi)

                def chunk_gates(i):
                    lo, hi = chunks[i]
                    gPs[i] = gpsum.tile([128, NGT, NC_MAX], f32, tag="gP", name=f"gP{d}_{i}")
                    emit_gates(gPs[i], usbs[i], hi - lo)

                def chunk_acts(i):
                    lo, hi = chunks[i]
                    ifos[i] = apool.tile([128, 6, NC_MAX], f16, tag="ifo", name=f"ifo{d}_{i}")
                    gts[i] = apool.tile([128, 2, NC_MAX], f16, tag="gt", name=f"gt{d}_{i}")
                    emit_acts(gPs[i], ifos[i], gts[i], hi - lo)

                def chunk_cell1(i):
                    lo, hi = chunks[i]
                    t1 = apool.tile([128, 2, NC_MAX], f16, tag="t1", name=f"t1_{d}_{i}")
                    emit_cell1(ifos[i], gts[i], t1, lo, hi)

                def chunk_tanhc(i):
                    lo, hi = chunks[i]
                    tc_t = apool.tile([128, 2, NC_MAX], f16, tag="tc", name=f"tc{d}_{i}")
                    nc.scalar.activation(tc_t[:, :, 0:hi - lo],
                                         cs[:, :, lo:hi], AF.Tanh)
                    return tc_t

                def chunk_cell2(i, tc_t):
                    lo, hi = chunks[i]
                    emit_cell2(ifos[i], tc_t, lo, hi)

                nch = len(chunks)
                if nch == 3:
                    chunk_u(0)
                    chunk_uadd(0)
                    chunk_gates(0)
                    chunk_u(1)
                    chunk_uadd(1)
                    chunk_acts(0)
                    chunk_cell1(0)
                    tc0 = chunk_tanhc(0)
                    chunk_cell2(0, tc0)
                    chunk_u(2)
                    chunk_uadd(2)
                    chunk_gates(1)
                    chunk_acts(1)
                    chunk_cell1(1)
                    tc1 = chunk_tanhc(1)
                    chunk_cell2(1, tc1)
                    chunk_gates(2)
                    chunk_acts(2)
                    chunk_cell1(2)
                    tc2 = chunk_tanhc(2)
                    chunk_cell2(2, tc2)
                else:
                    for i in range(nch):
                        chunk_u(i)
                        chunk_uadd(i)
                        chunk_gates(i)
                        chunk_acts(i)
                        chunk_cell1(i)
                        tci = chunk_tanhc(i)
                        chunk_cell2(i, tci)

                if d < nsteps - 1:
                    nc.gpsimd.tensor_scalar_mul(
                        h[:, :, OWN:OWN + 1], h[:, :, OWN:OWN + 1],
                        misc_t[:, 0:1])

                for kc in range(NKC):
                    nc.gpsimd.dma_start(hs_out[d, kc], h[:, kc, 0:OWN])

    nc.finalize()
    return nc


def _get_compiled(nsteps=WD):
    if nsteps not in _COMPILED:
        _COMPILED[nsteps] = _build(nsteps)
    return _COMPILED[nsteps]


# ------------------------------------------------------------------- driver

def kernel(x, w_i2s, b_i2s, w_ih, b_ih, b_hh, k0, k1, b_s2s):
    from concourse.bass_utils import run_bass_kernel_spmd

    in_maps = _prep_inputs(x, w_i2s, b_i2s, w_ih, b_ih, b_hh, k0, k1, b_s2s)
    nc = _get_compiled()
    res = run_bass_kernel_spmd(nc, in_maps, list(range(NCORES)))
    return _assemble([np.asarray(res.results[c]["hs"]).reshape(WD, CIN, OWN)
                      for c in range(NCORES)])


def kernel_numpy(x, w_i2s, b_i2s, w_ih, b_ih, b_hh, k0, k1, b_s2s):
    """Host-only replica of the device program (debug path)."""
    in_maps = _prep_inputs(x, w_i2s, b_i2s, w_ih, b_ih, b_hh, k0, k1, b_s2s)
    return _assemble([_core_sim(im).reshape(WD, CIN, OWN) for im in in_maps])


# revision 13
# speedup vs baseline: 1.8782x; 1.0852x over previous
"""DiagonalBiLSTM Trainium2 kernel, v2.

Full inputs in, full output out. Sharding: direction-parallel x row-parallel.
Cores 0-3 run the left-to-right diagonal scan over row blocks of 256
(4 batches each); cores 4-7 run the right-to-left scan (x W-flipped).
Each core computes 256 own rows plus a shrinking ghost region (126-d extra
rows at diagonal step d) so no inter-core communication is needed.

Per diagonal step d (n = 382 - d active columns, 3 column chunks <=128):
    u  = xs[d] + k0 @ h + k1 @ h(+1)        (xs = wm@x + b_i2s + b_s2s,
                                             precomputed on host, fp16)
    g  = w_ih @ u + bias                    (bias via K-hot bias matmuls)
    c  = sig(g_f) * c + sig(g_i) * tanh(g_g)
    h  = sig(g_o) * tanh(c)

All matmul operands fp16 (full-rate at any free dim, LDWEIGHTS hidden);
c state fp32. Column chunks are software-pipelined so the PE never idles
(avoids HAM clock-gate re-throttling) and Act/DVE run concurrently.
"""

import numpy as np

B, CIN, COUT, H, W, DC = 16, 256, 256, 64, 64, 3
WD = 2 * W - 1          # 127 diagonal steps
NCORES = 8
NDIRCORES = 4           # cores per direction
OWN = 256               # own rows per core (4 batches)
GHOST = 126
NDATA = OWN + GHOST     # 382 max active columns
NPAD = 384
NKC = 2                 # channel chunks (256 = 2*128)
NGT = 8                 # gate m-tiles (1024 = 8*128)
NC_MAX = 128            # max chunk width

# gate slot order in PSUM: [i0,i1,f0,f1,o0,o1,g0,g1] (w_ih row blocks)
SLOT_ROWS = [(0, 128), (128, 256), (256, 384), (384, 512),
             (768, 896), (896, 1024), (512, 640), (640, 768)]

_COMPILED = {}


# ----------------------------------------------------------------- host prep

def _i2s_mask_np():
    oc = np.arange(COUT) % DC
    ic = np.arange(CIN) % DC
    return (ic[None, :] <= oc[:, None]).astype(np.float32)


def _wT_tiles(w, nmt):
    # [out=nmt*128, in=256] -> lhsT tile array [k=128, kc=2, mt=nmt, m=128]
    return np.ascontiguousarray(
        w.T.reshape(NKC, 128, nmt, 128).transpose(1, 0, 2, 3))


def _prep_inputs(x, w_i2s, b_i2s, w_ih, b_ih, b_hh, k0, k1, b_s2s):
    x = np.asarray(x, np.float32)
    wm = (np.asarray(w_i2s, np.float32) * _i2s_mask_np())
    bias_u = (np.asarray(b_i2s, np.float32)
              + np.asarray(b_s2s, np.float32))          # [256]
    bias_g = (np.asarray(b_ih, np.float32)
              + np.asarray(b_hh, np.float32))           # [1024]

    k0_t = _wT_tiles(np.asarray(k0, np.float32), 2).astype(np.float16)
    k1_t = _wT_tiles(np.asarray(k1, np.float32), 2).astype(np.float16)

    # gate weight tiles in slot order: wih[k, kc, t, m]
    wih = np.empty((128, NKC, NGT, 128), np.float32)
    for t, (r0, r1) in enumerate(SLOT_ROWS):
        wt = _wT_tiles(np.asarray(w_ih, np.float32)[r0:r1], 1)  # [128,2,1,128]
        wih[:, :, t, :] = wt[:, :, 0, :]
    wih = wih.astype(np.float16)

    # bias stationaries: bgw[k, t, m] = bias_g[slot t, ch m] if k == 0
    bgw = np.zeros((128, NGT, 128), np.float32)
    for t, (r0, r1) in enumerate(SLOT_ROWS):
        bgw[0, t, :] = bias_g[r0:r1]
    bgw = bgw.astype(np.float16)

    # xs = wm @ x + bias_u for both directions: [B, 256, H, W]
    x2 = np.ascontiguousarray(x.transpose(1, 0, 2, 3)).reshape(CIN, -1)
    xs_l = np.ascontiguousarray(
        (wm @ x2).reshape(COUT, B, H, W).transpose(1, 0, 2, 3))
    xs_l += bias_u[None, :, None, None]
    xs_r = xs_l[:, :, :, ::-1]

    in_maps = []
    for core in range(NCORES):
        xs_d = xs_l if core < NDIRCORES else xs_r
        c0 = (core % NDIRCORES) * 4                     # first batch
        # rows r = 0..381 -> (b, h) = ((c0*64*4 + r)//64, r%64); rows beyond
        # B*H are bias-only (nonexistent ghost rows on the last core).
        xd = np.empty((WD, CIN, NPAD), np.float32)
        xd[:] = bias_u[None, :, None]                   # out-of-band fill
        nrows = min(NDATA, B * H - c0 * H)              # 382 or 256
        rb = np.arange(nrows)
        bs, hs = c0 + rb // H, rb % H
        # xd[d, :, r] = xs_d[b, :, h, d - h] when 0 <= d-h < W
        for h in range(H):
            sel = hs == h
            if not sel.any():
                continue
            rows = rb[sel]
            # steps d = h..h+W-1 map to w = 0..W-1
            blk = xs_d[bs[sel], :, h, :]                # [nr, 256, W]
            xd[h:h + W][:, :, rows] = blk.transpose(2, 1, 0)
        in_maps.append({
            "xd": np.ascontiguousarray(xd.reshape(WD, NKC, 128, NPAD)
                                       ).astype(np.float16),
            "k0t": k0_t, "k1t": k1_t, "wih": wih, "bgw": bgw,
            "misc": np.full((128, 4),
                            0.0 if core % NDIRCORES == NDIRCORES - 1 else 1.0,
                            np.float32),
        })
    return in_maps


# ----------------------------------------------------------- output assembly

def _assemble(core_outs):
    # core_outs: list of [WD, 2, 128, OWN] -> hs [2dir, WD, 256ch, 1024rows]
    hs = np.zeros((2, WD, CIN, B * H), np.float32)
    for c, o in enumerate(core_outs):
        d = c // NDIRCORES
        j = (c % NDIRCORES) * OWN
        hs[d, :, :, j:j + OWN] = np.asarray(o, np.float32).reshape(
            WD, CIN, OWN)

    def unscramble(hd):             # [WD, 256ch, 1024rows] -> [B, COUT, H, WD]
        a = hd.transpose(0, 2, 1).reshape(WD, B, COUT, H)
        return a.transpose(1, 2, 3, 0)

    def unshift(a):                 # [B, COUT, H, WD] -> [B, COUT, H, W]
        rows = np.arange(H)[:, None]
        cols = rows + np.arange(W)[None, :]
        return a[:, :, rows, cols]

    left = unshift(unscramble(hs[0]))
    right = unshift(unscramble(hs[1]))[:, :, :, ::-1]
    right = np.concatenate(
        [np.zeros_like(right[:, :, :1, :]), right[:, :, :-1, :]], axis=2)
    return left + right


# ------------------------------------------------------- reference-free host
# numpy replica of the device program (fp16 rounding modeled), for debugging

def _core_sim(im, nsteps=WD):
    f16 = np.float16
    xd = im["xd"]                        # [WD, 2, 128, NPAD] fp16

    def unT(t, nmt):                     # [k,kc,mt,m] -> [out, in] f32
        return np.float32(t).transpose(1, 0, 2, 3).reshape(
            CIN, nmt * 128).T

    k0 = unT(im["k0t"], 2)
    k1 = unT(im["k1t"], 2)
    wih_t = np.float32(im["wih"])        # [k, kc, t, m]
    bg = np.float32(im["bgw"])[0]        # [t, m]
    scale = im["misc"][0, 0]

    def sig(v):
        return 1.0 / (1.0 + np.exp(-v))

    h = np.zeros((CIN, NPAD), np.float32)     # ch x col, col 382+ stays 0
    c = np.zeros((CIN, NPAD), np.float32)
    out = np.zeros((nsteps, CIN, OWN), f16)
    for d in range(nsteps):
        n = NDATA - d
        xs = np.float32(xd[d].reshape(CIN, NPAD))[:, :n]
        u = f16(xs + k0 @ h[:, :n] + k1 @ h[:, 1:n + 1]).astype(np.float32)
        # gates per slot
        g = np.empty((NGT, 128, n), np.float32)
        for t in range(NGT):
            acc = bg[t][:, None] * np.ones((1, n), np.float32)
            for kc in range(NKC):
                acc = acc + wih_t[:, kc, t].T @ u[kc * 128:(kc + 1) * 128]
            g[t] = acc
        gi = np.concatenate([g[0], g[1]])
        gf = np.concatenate([g[2], g[3]])
        go = np.concatenate([g[4], g[5]])
        gg = np.concatenate([g[6], g[7]])
        t1 = f16(f16(sig(gi)) * f16(np.tanh(gg))).astype(np.float32)
        c[:, :n] = c[:, :n] * f16(sig(gf)).astype(np.float32) + t1
        h[:, :n] = f16(f16(sig(go)) * f16(np.tanh(c[:, :n]))).astype(
            np.float32)
        if d < WD - 1:
            h[:, OWN] *= scale
        out[d] = f16(h[:, :OWN])
    return out


# --------------------------------------------------------------- bass kernel

def _build(nsteps=WD):
    import concourse.bacc as bacc
    import concourse.mybir as mybir
    import concourse.tile as tile
    from concourse._compat import get_trn_type

    f32 = mybir.dt.float32
    f16 = mybir.dt.float16
    AF = mybir.ActivationFunctionType

    nc = bacc.Bacc(get_trn_type() or "TRN2", target_bir_lowering=False,
                   debug=False)
    xd = nc.dram_tensor("xd", [WD, NKC, 128, NPAD], f16, kind="ExternalInput")
    k0t = nc.dram_tensor("k0t", [128, NKC, 2, 128], f16, kind="ExternalInput")
    k1t = nc.dram_tensor("k1t", [128, NKC, 2, 128], f16, kind="ExternalInput")
    wih = nc.dram_tensor("wih", [128, NKC, NGT, 128], f16,
                         kind="ExternalInput")
    bgw = nc.dram_tensor("bgw", [128, NGT, 128], f16, kind="ExternalInput")
    misc = nc.dram_tensor("misc", [128, 4], f32, kind="ExternalInput")
    hs_out = nc.dram_tensor("hs", [WD, NKC, 128, OWN], f16,
                            kind="ExternalOutput")

    with tile.TileContext(nc) as tc:
        with (
            tc.tile_pool(name="wpool", bufs=1) as wpool,
            tc.tile_pool(name="state", bufs=1) as state,
            tc.tile_pool(name="xpool", bufs=4) as xpool,
            tc.tile_pool(name="upool", bufs=4) as upool,
            tc.tile_pool(name="apool", bufs=4) as apool,
            tc.tile_pool(name="upsum", bufs=2, space="PSUM") as upsum,
            tc.tile_pool(name="gpsum", bufs=3, space="PSUM") as gpsum,
        ):
            k0_t = wpool.tile([128, NKC, 2, 128], f16, tag="k0")
            k1_t = wpool.tile([128, NKC, 2, 128], f16, tag="k1")
            wih_t = wpool.tile([128, NKC, NGT, 128], f16, tag="wih")
            bgw_t = wpool.tile([128, NGT, 128], f16, tag="bgw")
            misc_t = wpool.tile([128, 4], f32, tag="misc")
            ones_t = wpool.tile([128, NC_MAX], f16, tag="ones")
            nc.sync.dma_start(k0_t[:], k0t[:])
            nc.sync.dma_start(k1_t[:], k1t[:])
            nc.sync.dma_start(wih_t[:], wih[:])
            nc.sync.dma_start(bgw_t[:], bgw[:])
            nc.sync.dma_start(misc_t[:], misc[:])
            nc.any.memset(ones_t[:], 1.0)

            h = state.tile([128, NKC, NPAD], f16, tag="h")
            cs = state.tile([128, NKC, NPAD], f32, tag="c")
            nc.any.memset(h[:], 0.0)
            nc.any.memset(cs[:], 0.0)

            PF = 2      # xs prefetch depth
            xs_tiles = {}
            for dd in range(min(PF + 1, nsteps)):
                t = xpool.tile([128, NKC, NPAD], f16, tag="xs", name=f"xs_pf")
                for kc in range(NKC):
                    nc.sync.dma_start(t[:, kc], xd[dd, kc])
                xs_tiles[dd] = t

            def emit_umm(uP, lo, hi):
                for m in range(NKC):
                    nc.tensor.matmul(uP[:, m, 0:hi - lo],
                                     k0_t[:, 0, m, :], h[:, 0, lo:hi],
                                     start=True, stop=False)
                    nc.tensor.matmul(uP[:, m, 0:hi - lo],
                                     k0_t[:, 1, m, :], h[:, 1, lo:hi],
                                     start=False, stop=False)
                    nc.tensor.matmul(uP[:, m, 0:hi - lo],
                                     k1_t[:, 0, m, :], h[:, 0, lo + 1:hi + 1],
                                     start=False, stop=False)
                    nc.tensor.matmul(uP[:, m, 0:hi - lo],
                                     k1_t[:, 1, m, :], h[:, 1, lo + 1:hi + 1],
                                     start=False, stop=True)

            def emit_gates(gP, u_sb, n):
                # slots 0-3 share a PSUM bank, 4-7 the other; start=True
                # clears has_written for the WHOLE bank, so only the first
                # bias matmul per bank may set it.
                for t in range(NGT):
                    nc.tensor.matmul(gP[:, t, 0:n], bgw_t[:, t, :],
                                     ones_t[:, 0:n], start=(t % 4 == 0),
                                     stop=False, skip_group_check=True)
                for t in range(NGT):
                    nc.tensor.matmul(gP[:, t, 0:n], wih_t[:, 0, t, :],
                                     u_sb[:, 0, 0:n], start=False, stop=False)
                for t in range(NGT):
                    nc.tensor.matmul(gP[:, t, 0:n], wih_t[:, 1, t, :],
                                     u_sb[:, 1, 0:n], start=False, stop=True)

            def emit_uadd(u_sb, uP, xs_t, lo, hi):
                for kc in range(NKC):
                    nc.vector.tensor_add(u_sb[:, kc, 0:hi - lo],
                                         uP[:, kc, 0:hi - lo],
                                         xs_t[:, kc, lo:hi])

            def emit_acts(gP, ifo, gt, n):
                nc.scalar.activation(ifo[:, :, 0:n], gP[:, 0:6, 0:n],
                                     AF.Sigmoid)
                nc.scalar.activation(gt[:, :, 0:n], gP[:, 6:8, 0:n], AF.Tanh)

            def emit_cell1(ifo, gt, t1, lo, hi):
                n = hi - lo
                nc.vector.tensor_mul(t1[:, :, 0:n], ifo[:, 0:2, 0:n],
                                     gt[:, :, 0:n])
                nc.vector.tensor_mul(cs[:, :, lo:hi], cs[:, :, lo:hi],
                                     ifo[:, 2:4, 0:n])
                nc.vector.tensor_add(cs[:, :, lo:hi], cs[:, :, lo:hi],
                                     t1[:, :, 0:n])

            def emit_cell2(ifo, tc_t, lo, hi):
                n = hi - lo
                nc.vector.tensor_mul(h[:, :, lo:hi], ifo[:, 4:6, 0:n],
                                     tc_t[:, :, 0:n])

            for d in range(nsteps):
                n = NDATA - d
                s1, s2 = 126 - d, 254 - d
                chunks = [(0, s1), (s1, s2), (s2, n)]
                chunks = [(lo, hi) for lo, hi in chunks if hi > lo]
                xs_t = xs_tiles.pop(d)
                if d + PF + 1 < nsteps:
                    t = xpool.tile([128, NKC, NPAD], f16, tag="xs", name=f"xs_pf")
                    for kc in range(NKC):
                        nc.sync.dma_start(t[:, kc], xd[d + PF + 1, kc])
                    xs_tiles[d + PF + 1] = t

                uPs, usbs, gPs, ifos, gts = {}, {}, {}, {}, {}

                def chunk_u(i):
                    lo, hi = chunks[i]
                    uPs[i] = upsum.tile([128, NKC, NC_MAX], f32, tag="uP", name=f"uP{d}_{i}")
                    emit_umm(uPs[i], lo, hi)

                def chunk_uadd(i):
                    lo, hi = chunks[i]
                    usbs[i] = upool.tile([128, NKC, NC_MAX], f16, tag="usb", name=f"usb{d}_{i}")
                    emit_uadd(usbs[i], uPs[i], xs_t, lo, hi)

                def chunk_gates(i):
                    lo, hi = chunks[i]
                    gPs[i] = gpsum.tile([128, NGT, NC_MAX], f32, tag="gP", name=f"gP{d}_{i}")
                    emit_gates(gPs[i], usbs[i], hi - lo)

                def chunk_acts(i):
                    lo, hi = chunks[i]
                    ifos[i] = apool.tile([128, 6, NC_MAX], f16, tag="ifo", name=f"ifo{d}_{i}")
                    gts[i] = apool.tile([128, 2, NC_MAX], f16, tag="gt", name=f"gt{d}_{i}")
                    emit_acts(gPs[i], ifos[i], gts[i], hi - lo)

                def chunk_cell1(i):
                    lo, hi = chunks[i]
                    t1 = apool.tile([128, 2, NC_MAX], f16, tag="t1", name=f"t1_{d}_{i}")
                    emit_cell1(ifos[i], gts[i], t1, lo, hi)

                def chunk_tanhc(i):
                    lo, hi = chunks[i]
                    tc_t = apool.tile([128, 2, NC_MAX], f16, tag="tc", name=f"tc{d}_{i}")
                    nc.scalar.activation(tc_t[:, :, 0:hi - lo],
                                         cs[:, :, lo:hi], AF.Tanh)
                    return tc_t

                def chunk_cell2(i, tc_t):
                    lo, hi = chunks[i]
                    emit_cell2(ifos[i], tc_t, lo, hi)

                nch = len(chunks)
                if nch == 3:
                    chunk_u(0)
                    chunk_uadd(0)
                    chunk_gates(0)
                    chunk_u(1)
                    chunk_uadd(1)
                    chunk_acts(0)
                    chunk_cell1(0)
                    tc0 = chunk_tanhc(0)
                    chunk_cell2(0, tc0)
                    chunk_u(2)
                    chunk_uadd(2)
                    chunk_gates(1)
                    chunk_acts(1)
                    chunk_cell1(1)
                    tc1 = chunk_tanhc(1)
                    chunk_cell2(1, tc1)
                    chunk_gates(2)
                    chunk_acts(2)
                    chunk_cell1(2)
                    tc2 = chunk_tanhc(2)
                    chunk_cell2(2, tc2)
                else:
                    for i in range(nch):
                        chunk_u(i)
                        chunk_uadd(i)
                        chunk_gates(i)
                        chunk_acts(i)
                        chunk_cell1(i)
                        tci = chunk_tanhc(i)
                        chunk_cell2(i, tci)

                if d < nsteps - 1:
                    nc.gpsimd.tensor_scalar_mul(
                        h[:, :, OWN:OWN + 1], h[:, :, OWN:OWN + 1],
                        misc_t[:, 0:1])

                for kc in range(NKC):
                    nc.gpsimd.dma_start(hs_out[d, kc], h[:, kc, 0:OWN])

    nc.finalize()
    return nc


def _get_compiled(nsteps=WD):
    if nsteps not in _COMPILED:
        _COMPILED[nsteps] = _build(nsteps)
    return _COMPILED[nsteps]


# ------------------------------------------------------------------- driver

def kernel(x, w_i2s, b_i2s, w_ih, b_ih, b_hh, k0, k1, b_s2s):
    from concourse.bass_utils import run_bass_kernel_spmd

    in_maps = _prep_inputs(x, w_i2s, b_i2s, w_ih, b_ih, b_hh, k0, k1, b_s2s)
    nc = _get_compiled()
    res = run_bass_kernel_spmd(nc, in_maps, list(range(NCORES)))
    return _assemble([np.asarray(res.results[c]["hs"]).reshape(WD, CIN, OWN)
                      for c in range(NCORES)])


def kernel_numpy(x, w_i2s, b_i2s, w_ih, b_ih, b_hh, k0, k1, b_s2s):
    """Host-only replica of the device program (debug path)."""
    in_maps = _prep_inputs(x, w_i2s, b_i2s, w_ih, b_ih, b_hh, k0, k1, b_s2s)
    return _assemble([_core_sim(im).reshape(WD, CIN, OWN) for im in in_maps])
